# revision 41
# baseline (speedup 1.0000x reference)
"""Trainium2 Bass kernel for per-pixel temporal attention (nn_Attention).

Reference computation, per pixel (B,H,W independent; T=8, C=3):
  x = Linear_in(z); q,k,v = Linear_{q,k,v}(x); 4-head attention over T,
  take row t=T-1, project to 3 channels.

Only the LAST timestep's attention output is used, so the whole pipeline
folds (host-side, weights only) to per-pixel:
  yq[h,d] = sum_{c'} G'[h,c',d]*z7'[c']          (z7' = [z7; 1], 12 outs)
  s[h,t]  = sum_d yq[h,d]*z[t,d]                 (32)
  e = exp(s); den[h] = sum_t e; r = 1/den
  zbar[h,d] = sum_t e[h,t]*z[t,d]
  out[c] = sum_{h,d} M[h,c,d]*(zbar[h,d]*r[h]) + bhat[c]

Sharding: data-parallel over 8 cores; core i takes batch b=i//2,
row-half i%2 -> a (24, 32768) shard per core; z uploaded as fp16.

Device mapping (pixels-on-partitions: 128 partitions x 256 pixels,
per-pixel features as fp16 planes of 256 on the free axis):
  - products (yq*z, e*z, zbar*r)          -> VectorE fp16 TT (2x mode)
  - yq / s-sum / zbar-sum / M-map         -> TensorE scaled-identity
    matmuls accumulating in PSUM fp32
  - exp, PSUM evictions                   -> ScalarE (ACT)
  - den accumulation                      -> GpSimd (Pool), in SBUF fp16
  - reciprocal                            -> VectorE
Output written fp16, host casts to fp32.
"""

import numpy as np

HEADS, DK = 4, 8
B, H, W = 4, 256, 256
NPIX = 128 * 256          # pixels per core shard
NF = 256                  # pixels per partition
NCORES = 8

_CACHE = {}


def _fold_weights(W_in, b_in, W_q, b_q, W_k, b_k, W_v, b_v, W_o, b_o):
    f8 = np.float64
    W_in, b_in, W_q, b_q, W_k, b_k, W_v, b_v, W_o, b_o = [
        np.asarray(x, f8) for x in (W_in, b_in, W_q, b_q, W_k, b_k, W_v, b_v, W_o, b_o)]
    A_q = W_q @ W_in; c_q = W_q @ b_in + b_q
    A_k = W_k @ W_in; c_k = W_k @ b_in + b_k
    A_v = W_v @ W_in; c_v = W_v @ b_in + b_v
    scale = 1.0 / np.sqrt(DK)
    Ghat = np.zeros((HEADS, 3, 3)); ghat = np.zeros((HEADS, 3)); M = np.zeros((HEADS, 3, 3))
    for h in range(HEADS):
        sl = slice(h * DK, (h + 1) * DK)
        Ghat[h] = A_q[sl].T @ A_k[sl] * scale
        ghat[h] = A_k[sl].T @ c_q[sl] * scale
        M[h] = W_o[:, sl] @ A_v[sl]
    bhat = W_o @ c_v + b_o
    return (Ghat.astype(np.float32), ghat.astype(np.float32),
            M.astype(np.float32), bhat.astype(np.float32))


# Stacked 128x128 fp16 weight mats: [0]=I, [1+j*4+c'] = G'[h,c',d]*I for
# j=(h*3+d), c'=0..3 (c'=3 is the ghat term fed by a ones plane),
# [49+h*9+c*3+d] = M[h,c,d]*I.
NMATS = 85

# PE p-state bridge sizes (64-col dummy matmuls across known PE idle
# windows; tuned against the cost-model timeline)
N_BRIDGE_HEAD = 20
N_BRIDGE_TAIL = 0
DEBUG_TAPS = False


def _build_program():
    import concourse.bass as bass
    import concourse.tile as tile
    from concourse import bacc, mybir

    f32, f16 = mybir.dt.float32, mybir.dt.float16
    MULT, ADD = mybir.AluOpType.mult, mybir.AluOpType.add
    ACTF = mybir.ActivationFunctionType

    nc = bacc.Bacc("TRN2", target_bir_lowering=False, debug=False)
    z_dram = nc.dram_tensor("z", [24, NPIX], f16, kind="ExternalInput").ap()
    i_dram = nc.dram_tensor("ident", [128, NMATS * 128], f16, kind="ExternalInput").ap()
    c_dram = nc.dram_tensor("consts", [128, 4], f32, kind="ExternalInput").ap()
    o_dram = nc.dram_tensor("out", [3, NPIX], f16, kind="ExternalOutput").ap()
    if DEBUG_TAPS:
        dbg_yq = nc.dram_tensor("dbg_yq", [128, 12 * NF], f16, kind="ExternalOutput").ap()
        dbg_e = nc.dram_tensor("dbg_e", [128, 32 * NF], f16, kind="ExternalOutput").ap()
        dbg_den = nc.dram_tensor("dbg_den", [128, 4 * NF], f16, kind="ExternalOutput").ap()
        dbg_zb = nc.dram_tensor("dbg_zb", [128, 12 * NF], f16, kind="ExternalOutput").ap()

    with tile.TileContext(nc) as tc:
        with (
            tc.tile_pool(name="const", bufs=1) as cpool,
            tc.tile_pool(name="z16", bufs=1) as z16pool,
            tc.tile_pool(name="work", bufs=1) as wpool,
            tc.tile_pool(name="prod", bufs=1) as ppool,
            tc.tile_pool(name="psum", bufs=1, space="PSUM") as pspool,
            tc.tile_pool(name="piece", bufs=2, space="PSUM") as piecepool,
        ):
            wmats = cpool.tile([128, NMATS * 128], f16)
            ident = wmats[:, 0:128]

            def wG(h, d, cp):   # G'[h,c',d] * I
                j = 1 + (h * 3 + d) * 4 + cp
                return wmats[:, j * 128:(j + 1) * 128]

            def wM(h, c, d):    # M[h,c,d] * I
                j = 49 + h * 9 + c * 3 + d
                return wmats[:, j * 128:(j + 1) * 128]

            zsrc = z_dram.rearrange("(t c) (p n) -> p t c n", t=8, c=3, p=128)
            z16 = z16pool.tile([128, 24 * NF], f16)
            z16v = z16.rearrange("p (t c n) -> p t c n", t=8, c=3)

            # ---- DMA order: ident+z7 first, then G' mats in per-h chunks,
            #      then the rest of z, then M mats; consts (only needed at
            #      the end) last. Two issue queues overlap per-DMA setup.
            nc.sync.dma_start(out=wmats[:, 0:128], in_=i_dram[:, 0:128])
            nc.scalar.dma_start(out=z16v[:, 7, :, :], in_=zsrc[:, 7, :, :])
            for h in range(4):
                lo, hi = (1 + h * 12) * 128, (13 + h * 12) * 128
                nc.sync.dma_start(out=wmats[:, lo:hi], in_=i_dram[:, lo:hi])
            nc.scalar.dma_start(out=z16v[:, 0:4, :, :], in_=zsrc[:, 0:4, :, :])
            nc.scalar.dma_start(out=z16v[:, 4:7, :, :], in_=zsrc[:, 4:7, :, :])
            nc.sync.dma_start(out=wmats[:, 49 * 128:], in_=i_dram[:, 49 * 128:])
            consts = cpool.tile([128, 4], f32)
            nc.scalar.dma_start(out=consts[:], in_=c_dram)

            # ones plane for the ghat (c'=3) yq terms
            ones = wpool.tile([128, NF], f16, tag="ones")
            nc.gpsimd.memset(ones[:], 1.0)

            def bridge(n):
                """Keep the PE p-state ramp alive across known idle windows:
                tiny matmuls into a rotating piece, no data deps beyond ident."""
                for _ in range(n):
                    dps = piecepool.tile([128, 2 * NF], f32, tag="piece")
                    nc.tensor.matmul(dps[:, 0:64], ident,
                                     ident[:, 0:64], start=True, stop=True)

            bridge(N_BRIDGE_HEAD)

            # ---- yq[h,d] = sum_c' G'[h,c',d]*z7'[c']  (48 scaled-I
            #      matmuls), h-pair at a time with interleaved eviction so
            #      the P products can start after the first pair.
            z7p = [z16v[:, 7, c, :] for c in range(3)] + [ones[:]]
            yq_ps = pspool.tile([128, 12 * NF], f32, tag="big")
            yq16 = wpool.tile([128, 12 * NF], f16, tag="yq16")
            for h in range(4):
                for d in range(3):
                    j = h * 3 + d
                    for cp in range(4):
                        nc.tensor.matmul(yq_ps[:, j * NF:(j + 1) * NF],
                                         wG(h, d, cp), z7p[cp],
                                         start=(cp == 0), stop=(cp == 3))
                nc.scalar.activation(yq16[:, h * 3 * NF:(h + 1) * 3 * NF],
                                     yq_ps[:, h * 3 * NF:(h + 1) * 3 * NF],
                                     ACTF.Identity, bias=0.0)
            yqv = yq16.rearrange("p (h d n) -> p h d n", h=4, d=3)

            # ---- t-pipelined middle: P (DVE) -> s (PE) -> exp (ACT)
            #      -> P2 (DVE) -> zbar accumulation (PE), den accum (Pool)
            P = ppool.tile([128, 96 * NF], f16, tag="P")
            Pv = P.rearrange("p (t d h n) -> p t d h n", t=8, d=3, h=4)
            P2 = ppool.tile([128, 96 * NF], f16, tag="P2")
            # (t, hpair, d, h2, n): the (d,h2) slab per (t,hpair) is contiguous
            P2v = P2.rearrange("p (t k d h n) -> p t k d h n", t=8, k=2, d=3, h=2)
            E = wpool.tile([128, 32 * NF], f16, tag="E")
            Ev = E.rearrange("p (t h n) -> p t h n", t=8, h=4)
            den16 = wpool.tile([128, 4 * NF], f16, tag="den16")
            r16 = wpool.tile([128, 4 * NF], f16, tag="r16")
            zb_ps = pspool.tile([128, 12 * NF], f32, tag="big")
            # layout (k, d, h2): the per-h-pair slab [p, k] is contiguous
            zbpv = zb_ps.rearrange("p (k d h n) -> p k d h n", k=2, d=3, h=2)
            zb16 = wpool.tile([128, 12 * NF], f16, tag="zb16")
            zbv = zb16.rearrange("p (k d h n) -> p k d h n", k=2, d=3, h=2)
            yb = yqv.transpose([0, 2, 1, 3]).unsqueeze(1).broadcast_to((128, 1, 3, 4, NF))

            for t in range(8):
                zbt = z16v[:, t:t + 1, :, :].unsqueeze(3).broadcast_to((128, 1, 3, 4, NF))
                for k in range(2):
                    hs = slice(2 * k, 2 * k + 2)
                    # P[t,d,h-half] = yq[h,d] * z[t,d]   (DVE fp16 2x)
                    nc.vector.tensor_tensor(Pv[:, t:t + 1, :, hs, :],
                                            yb[:, :, :, hs, :],
                                            zbt[:, :, :, 0:2, :], MULT)
                    # s piece (h-pair) -> exp
                    s_ps = piecepool.tile([128, 2 * NF], f32, tag="piece")
                    for d in range(3):
                        nc.tensor.matmul(s_ps[:], ident,
                                         Pv[:, t, d, hs, :],
                                         start=(d == 0), stop=(d == 2))
                    nc.scalar.activation(Ev[:, t, hs, :],
                                         s_ps.rearrange("p (j n) -> p j n", j=2),
                                         ACTF.Exp)
                    # P2[t,k,d,h2] = e[t,h] * z[t,d]   (DVE fp16 2x)
                    ebt = Ev[:, t:t + 1, hs, :].unsqueeze(2).broadcast_to((128, 1, 3, 2, NF))
                    nc.vector.tensor_tensor(P2v[:, t:t + 1, k, :, :, :], ebt,
                                            zbt[:, :, :, 0:2, :], MULT)
                    # zbar accumulation: contiguous 2-plane (one-bank) pieces
                    off = (t * 2 + k) * 6 * NF
                    for q in range(3):
                        dst = (k * 6 + q * 2) * NF
                        nc.tensor.matmul(zb_ps[:, dst:dst + 2 * NF], ident,
                                         P2[:, off + q * 2 * NF:off + (q + 1) * 2 * NF],
                                         start=(t == 0), stop=(t == 7))
                    if t == 7:
                        # this h-pair's zbar groups just stopped: evict now so
                        # the tail's zr/Mout chain starts as early as possible
                        for d in range(3):
                            nc.scalar.activation(zbv[:, k, d:d + 1, :, :],
                                                 zbpv[:, k, d:d + 1, :, :],
                                                 ACTF.Identity, bias=0.0)
                        # finish this pair's denominator + reciprocal on DVE
                        dsl = slice(2 * k * NF, (2 * k + 2) * NF)
                        nc.vector.tensor_tensor(den16[:, dsl], den16[:, dsl],
                                                Ev[:, 7, hs, :], ADD)
                        with nc.allow_low_precision(reason="fp16 r is fine"):
                            nc.vector.reciprocal(r16[:, dsl], den16[:, dsl])
                    # den accumulation on Pool (SBUF fp16), per h-pair chain
                    elif t == 0:
                        nc.gpsimd.tensor_copy(den16[:, 2 * k * NF:(2 * k + 2) * NF],
                                              Ev[:, 0, hs, :])
                    else:
                        dsl = slice(2 * k * NF, (2 * k + 2) * NF)
                        nc.gpsimd.tensor_tensor(den16[:, dsl], den16[:, dsl],
                                                Ev[:, t, hs, :], ADD)

            bridge(N_BRIDGE_TAIL)
            r16v = r16.rearrange("p (h n) -> p h n", h=4)
            zr = wpool.tile([128, 12 * NF], f16, tag="zr")
            zrv = zr.rearrange("p (k d h n) -> p k d h n", k=2, d=3, h=2)

            # PSUM accumulate-groups are clobbered by any start=True on the
            # same bank -> one group per bank: c0/c1 in the two piece bufs,
            # c2 in the (freed) big-pool tile.
            out_ps0 = piecepool.tile([128, 2 * NF], f32, tag="piece")
            out_ps1 = piecepool.tile([128, 2 * NF], f32, tag="piece")
            out_ps2 = pspool.tile([128, 12 * NF], f32, tag="big")

            def out_slot(c):
                return [out_ps0, out_ps1, out_ps2][c][:, 0:NF]

            # fine-grained tail pipeline: per (k,d) scale -> matmuls
            for k in range(2):
                hs = slice(2 * k, 2 * k + 2)
                rbk = r16v[:, hs, :].unsqueeze(0 + 1).broadcast_to((128, 1, 2, NF))
                for d in range(3):
                    nc.vector.tensor_tensor(zrv[:, k, d:d + 1, :, :],
                                            zbv[:, k, d:d + 1, :, :], rbk, MULT)
                    for c in range(3):
                        for h2 in range(2):
                            nc.tensor.matmul(out_slot(c),
                                             wM(2 * k + h2, c, d),
                                             zrv[:, k, d, h2, :],
                                             start=(k == 0 and d == 0 and h2 == 0),
                                             stop=(k == 1 and d == 2 and h2 == 1))
            out16 = wpool.tile([128, 3 * NF], f16, tag="out16")
            odst = o_dram.rearrange("c (p n) -> p c n", p=128)
            for c in range(3):
                nc.scalar.activation(out16[:, c * NF:(c + 1) * NF],
                                     out_slot(c),
                                     ACTF.Identity, bias=consts[:, c:c + 1])
                nc.sync.dma_start(out=odst[:, c:c + 1, :],
                                  in_=out16[:, c * NF:(c + 1) * NF])
            if DEBUG_TAPS:
                nc.sync.dma_start(out=dbg_yq, in_=yq16[:])
                nc.sync.dma_start(out=dbg_e, in_=E[:])
                nc.sync.dma_start(out=dbg_den, in_=r16[:])
                nc.sync.dma_start(out=dbg_zb, in_=zr[:])

    nc.finalize()
    return nc


def _get_program(key):
    if key not in _CACHE:
        _CACHE[key] = _build_program()
    return _CACHE[key]


def _weight_mats_f16(Ghat, ghat, M):
    """85 stacked 128x128 fp16 mats: [0]=I, [1:49]=G'*I, [49:85]=M*I."""
    eye = np.eye(128, dtype=np.float32)
    mats = np.empty((NMATS, 128, 128), np.float32)
    mats[0] = eye
    Gp = np.empty((HEADS, 3, 4), np.float32)   # (h, d, c')
    Gp[:, :, :3] = np.transpose(Ghat, (0, 2, 1))  # G'[h,d,c] = Ghat[h,c,d]
    Gp[:, :, 3] = ghat
    mats[1:49] = Gp.reshape(48, 1, 1) * eye
    mats[49:85] = M.reshape(36, 1, 1) * eye
    return np.ascontiguousarray(
        mats.transpose(1, 0, 2).reshape(128, NMATS * 128)).astype(np.float16)


def kernel(z_receive, W_in, b_in, W_q, b_q, W_k, b_k, W_v, b_v, W_o, b_o):
    from concourse.bass_utils import run_bass_kernel_spmd

    z_receive = np.asarray(z_receive, np.float32)
    Ghat, ghat, M, bhat = _fold_weights(W_in, b_in, W_q, b_q, W_k, b_k, W_v, b_v, W_o, b_o)
    wm = _weight_mats_f16(Ghat, ghat, M)
    consts = np.zeros((128, 4), np.float32)
    consts[:, 0:3] = bhat

    nc = _get_program("trn2_attn_v3")

    z16 = z_receive.astype(np.float16)
    in_maps = []
    for i in range(NCORES):
        b, hh = i // 2, (i % 2) * 128
        shard = np.ascontiguousarray(
            z16[b, :, :, hh:hh + 128, :]).reshape(24, NPIX)
        in_maps.append({"z": shard, "ident": wm, "consts": consts})

    res = run_bass_kernel_spmd(nc, in_maps, list(range(NCORES)))

    out = np.empty((B, 3, H, W), np.float32)
    for i in range(NCORES):
        b, hh = i // 2, (i % 2) * 128
        out[b, :, hh:hh + 128, :] = res.results[i]["out"].astype(np.float32).reshape(3, 128, W)
    return out


# revision 45
# speedup vs baseline: 1.0515x; 1.0515x over previous
"""Trainium2 Bass kernel for per-pixel temporal attention (nn_Attention).

Reference computation, per pixel (B,H,W independent; T=8, C=3):
  x = Linear_in(z); q,k,v = Linear_{q,k,v}(x); 4-head attention over T,
  take row t=T-1, project to 3 channels.

Only the LAST timestep's attention output is used, so the whole pipeline
folds (host-side, weights only) to per-pixel:
  yq[h,d] = sum_{c'} G'[h,c',d]*z7'[c']          (z7' = [z7; 1], 12 outs)
  s[h,t]  = sum_d yq[h,d]*z[t,d]                 (32)
  e = exp(s); den[h] = sum_t e; r = 1/den
  zbar[h,d] = sum_t e[h,t]*z[t,d]
  out[c] = sum_{h,d} M[h,c,d]*(zbar[h,d]*r[h]) + bhat[c]

Sharding: data-parallel over 8 cores; core i takes batch b=i//2,
row-half i%2 -> a (24, 32768) shard per core; z uploaded as fp16.

Device mapping (pixels-on-partitions: 128 partitions x 256 pixels,
per-pixel features as fp16 planes of 256 on the free axis):
  - products (yq*z, e*z, zbar*r)          -> VectorE fp16 TT (2x mode)
  - yq / s-sum / zbar-sum / M-map         -> TensorE scaled-identity
    matmuls accumulating in PSUM fp32
  - exp, PSUM evictions                   -> ScalarE (ACT)
  - den accumulation                      -> GpSimd (Pool), in SBUF fp16
  - reciprocal                            -> VectorE
Output written fp16, host casts to fp32.
"""

import numpy as np

HEADS, DK = 4, 8
B, H, W = 4, 256, 256
NPIX = 128 * 256          # pixels per core shard
NF = 256                  # pixels per partition
NCORES = 8

_CACHE = {}


def _fold_weights(W_in, b_in, W_q, b_q, W_k, b_k, W_v, b_v, W_o, b_o):
    f8 = np.float64
    W_in, b_in, W_q, b_q, W_k, b_k, W_v, b_v, W_o, b_o = [
        np.asarray(x, f8) for x in (W_in, b_in, W_q, b_q, W_k, b_k, W_v, b_v, W_o, b_o)]
    A_q = W_q @ W_in; c_q = W_q @ b_in + b_q
    A_k = W_k @ W_in; c_k = W_k @ b_in + b_k
    A_v = W_v @ W_in; c_v = W_v @ b_in + b_v
    scale = 1.0 / np.sqrt(DK)
    Ghat = np.zeros((HEADS, 3, 3)); ghat = np.zeros((HEADS, 3)); M = np.zeros((HEADS, 3, 3))
    for h in range(HEADS):
        sl = slice(h * DK, (h + 1) * DK)
        Ghat[h] = A_q[sl].T @ A_k[sl] * scale
        ghat[h] = A_k[sl].T @ c_q[sl] * scale
        M[h] = W_o[:, sl] @ A_v[sl]
    bhat = W_o @ c_v + b_o
    return (Ghat.astype(np.float32), ghat.astype(np.float32),
            M.astype(np.float32), bhat.astype(np.float32))


# Stacked 128x128 fp16 weight mats: [0]=I; [1+((h-2)*3+d)*4+c'] =
# G'[h,c',d]*I for h=2,3 only (heads 0,1 compute yq on DVE with scalar
# immediates); [25+h*9+c*3+d] = M[h,c,d]*I.
NMATS = 61

# folded weights baked into the program as immediates (set by kernel())
_GHAT = None
_GSMALL = None

# PE p-state bridge sizes (64-col dummy matmuls across known PE idle
# windows; tuned against the cost-model timeline)
N_BRIDGE_HEAD = 40
N_BRIDGE_TAIL = 0
DEBUG_TAPS = False


def _build_program():
    import concourse.bass as bass
    import concourse.tile as tile
    from concourse import bacc, mybir

    f32, f16 = mybir.dt.float32, mybir.dt.float16
    MULT, ADD = mybir.AluOpType.mult, mybir.AluOpType.add
    ACTF = mybir.ActivationFunctionType

    nc = bacc.Bacc("TRN2", target_bir_lowering=False, debug=False)
    z_dram = nc.dram_tensor("z", [24, NPIX], f16, kind="ExternalInput").ap()
    i_dram = nc.dram_tensor("ident", [128, NMATS * 128], f16, kind="ExternalInput").ap()
    c_dram = nc.dram_tensor("consts", [128, 4], f32, kind="ExternalInput").ap()
    o_dram = nc.dram_tensor("out", [3, NPIX], f16, kind="ExternalOutput").ap()
    if DEBUG_TAPS:
        dbg_yq = nc.dram_tensor("dbg_yq", [128, 12 * NF], f16, kind="ExternalOutput").ap()
        dbg_e = nc.dram_tensor("dbg_e", [128, 32 * NF], f16, kind="ExternalOutput").ap()
        dbg_den = nc.dram_tensor("dbg_den", [128, 4 * NF], f16, kind="ExternalOutput").ap()
        dbg_zb = nc.dram_tensor("dbg_zb", [128, 12 * NF], f16, kind="ExternalOutput").ap()

    with tile.TileContext(nc) as tc:
        with (
            tc.tile_pool(name="const", bufs=1) as cpool,
            tc.tile_pool(name="z16", bufs=1) as z16pool,
            tc.tile_pool(name="work", bufs=1) as wpool,
            tc.tile_pool(name="prod", bufs=1) as ppool,
            tc.tile_pool(name="psum", bufs=1, space="PSUM") as pspool,
            tc.tile_pool(name="piece", bufs=2, space="PSUM") as piecepool,
        ):
            wmats = cpool.tile([128, NMATS * 128], f16)
            ident = wmats[:, 0:128]

            def wG(h, d, cp):   # G'[h,c',d] * I   (h = 2 or 3)
                j = 1 + ((h - 2) * 3 + d) * 4 + cp
                return wmats[:, j * 128:(j + 1) * 128]

            def wM(h, c, d):    # M[h,c,d] * I
                j = 25 + h * 9 + c * 3 + d
                return wmats[:, j * 128:(j + 1) * 128]

            zsrc = z_dram.rearrange("(t c) (p n) -> p t c n", t=8, c=3, p=128)
            z16 = z16pool.tile([128, 24 * NF], f16)
            z16v = z16.rearrange("p (t c n) -> p t c n", t=8, c=3)

            # ---- DMA order: ident+z7 first, then G' mats in per-h chunks,
            #      then the rest of z, then M mats; consts (only needed at
            #      the end) last. Two issue queues overlap per-DMA setup.
            nc.sync.dma_start(out=wmats[:, 0:128], in_=i_dram[:, 0:128])
            nc.scalar.dma_start(out=z16v[:, 7, :, :], in_=zsrc[:, 7, :, :])
            for hh in range(2):
                lo, hi = (1 + hh * 12) * 128, (13 + hh * 12) * 128
                nc.sync.dma_start(out=wmats[:, lo:hi], in_=i_dram[:, lo:hi])
            nc.scalar.dma_start(out=z16v[:, 0:4, :, :], in_=zsrc[:, 0:4, :, :])
            nc.scalar.dma_start(out=z16v[:, 4:7, :, :], in_=zsrc[:, 4:7, :, :])
            nc.sync.dma_start(out=wmats[:, 25 * 128:], in_=i_dram[:, 25 * 128:])
            consts = cpool.tile([128, 4], f32)
            nc.scalar.dma_start(out=consts[:], in_=c_dram)

            # ones plane for the ghat (c'=3) yq terms
            ones = wpool.tile([128, NF], f16, tag="ones")
            nc.gpsimd.memset(ones[:], 1.0)

            def bridge(n):
                """Keep the PE p-state ramp alive across known idle windows:
                tiny matmuls into a rotating piece, no data deps beyond ident."""
                for _ in range(n):
                    dps = piecepool.tile([128, 2 * NF], f32, tag="piece")
                    nc.tensor.matmul(dps[:, 0:64], ident,
                                     ident[:, 0:64], start=True, stop=True)

            bridge(N_BRIDGE_HEAD)

            # ---- yq[h,d] = sum_c Ghat*z7[c] + ghat: heads 0,1 on DVE
            #      (tensor_scalar 4x with immediates, no weight-DMA wait) so
            #      the t-loop starts at z7-arrival; heads 2,3 on PE+ACT in
            #      parallel via the scaled-identity route.
            z7p = [z16v[:, 7, c, :] for c in range(3)]
            yq16 = wpool.tile([128, 12 * NF], f16, tag="yq16")
            yqv = yq16.rearrange("p (h d n) -> p h d n", h=4, d=3)
            sz = wpool.tile([128, 9 * NF], f16, tag="sz")
            szv = sz.rearrange("p (c d n) -> p c d n", c=3, d=3)
            tmp = wpool.tile([128, 3 * NF], f16, tag="yqtmp")
            tmpv = tmp.rearrange("p (d n) -> p d n", d=3)
            for h in range(2):
                for c in range(3):
                    for d in range(3):
                        s2 = float(_GSMALL[h, d]) if c == 0 else 0.0
                        nc.vector.tensor_scalar(
                            szv[:, c, d, :], z7p[c],
                            float(_GHAT[h, c, d]), s2, MULT, ADD)
                nc.vector.tensor_tensor(tmpv[:, :, :], szv[:, 0, :, :],
                                        szv[:, 1, :, :], ADD)
                nc.vector.tensor_tensor(yqv[:, h, :, :], tmpv[:, :, :],
                                        szv[:, 2, :, :], ADD)
            z7q = z7p + [ones[:]]
            yq_ps = pspool.tile([128, 12 * NF], f32, tag="big")
            for h in (2, 3):
                for d in range(3):
                    j = h * 3 + d
                    for cp in range(4):
                        nc.tensor.matmul(yq_ps[:, j * NF:(j + 1) * NF],
                                         wG(h, d, cp), z7q[cp],
                                         start=(cp == 0), stop=(cp == 3))
            nc.scalar.activation(yq16[:, 6 * NF:], yq_ps[:, 6 * NF:],
                                 ACTF.Identity, bias=0.0)
            yqv = yq16.rearrange("p (h d n) -> p h d n", h=4, d=3)

            # ---- t-pipelined middle: P (DVE) -> s (PE) -> exp (ACT)
            #      -> P2 (DVE) -> zbar accumulation (PE), den accum (Pool)
            P = ppool.tile([128, 96 * NF], f16, tag="P")
            Pv = P.rearrange("p (t d h n) -> p t d h n", t=8, d=3, h=4)
            P2 = ppool.tile([128, 96 * NF], f16, tag="P2")
            # (t, hpair, d, h2, n): the (d,h2) slab per (t,hpair) is contiguous
            P2v = P2.rearrange("p (t k d h n) -> p t k d h n", t=8, k=2, d=3, h=2)
            E = wpool.tile([128, 32 * NF], f16, tag="E")
            Ev = E.rearrange("p (t h n) -> p t h n", t=8, h=4)
            den16 = wpool.tile([128, 4 * NF], f16, tag="den16")
            r16 = wpool.tile([128, 4 * NF], f16, tag="r16")
            zb_ps = pspool.tile([128, 12 * NF], f32, tag="big")
            # layout (k, d, h2): the per-h-pair slab [p, k] is contiguous
            zbpv = zb_ps.rearrange("p (k d h n) -> p k d h n", k=2, d=3, h=2)
            zb16 = wpool.tile([128, 12 * NF], f16, tag="zb16")
            zbv = zb16.rearrange("p (k d h n) -> p k d h n", k=2, d=3, h=2)
            yb = yqv.transpose([0, 2, 1, 3]).unsqueeze(1).broadcast_to((128, 1, 3, 4, NF))

            for t in range(8):
                zbt = z16v[:, t:t + 1, :, :].unsqueeze(3).broadcast_to((128, 1, 3, 4, NF))
                for k in range(2):
                    hs = slice(2 * k, 2 * k + 2)
                    # P[t,d,h-half] = yq[h,d] * z[t,d]   (DVE fp16 2x)
                    nc.vector.tensor_tensor(Pv[:, t:t + 1, :, hs, :],
                                            yb[:, :, :, hs, :],
                                            zbt[:, :, :, 0:2, :], MULT)
                    # s piece (h-pair) -> exp
                    s_ps = piecepool.tile([128, 2 * NF], f32, tag="piece")
                    for d in range(3):
                        nc.tensor.matmul(s_ps[:], ident,
                                         Pv[:, t, d, hs, :],
                                         start=(d == 0), stop=(d == 2))
                    nc.scalar.activation(Ev[:, t, hs, :],
                                         s_ps.rearrange("p (j n) -> p j n", j=2),
                                         ACTF.Exp)
                    # P2[t,k,d,h2] = e[t,h] * z[t,d]   (DVE fp16 2x)
                    ebt = Ev[:, t:t + 1, hs, :].unsqueeze(2).broadcast_to((128, 1, 3, 2, NF))
                    nc.vector.tensor_tensor(P2v[:, t:t + 1, k, :, :, :], ebt,
                                            zbt[:, :, :, 0:2, :], MULT)
                    # zbar accumulation: contiguous 2-plane (one-bank) pieces
                    off = (t * 2 + k) * 6 * NF
                    for q in range(3):
                        dst = (k * 6 + q * 2) * NF
                        nc.tensor.matmul(zb_ps[:, dst:dst + 2 * NF], ident,
                                         P2[:, off + q * 2 * NF:off + (q + 1) * 2 * NF],
                                         start=(t == 0), stop=(t == 7))
                    if t == 7:
                        # this h-pair's zbar groups just stopped: evict now so
                        # the tail's zr/Mout chain starts as early as possible
                        for d in range(3):
                            nc.scalar.activation(zbv[:, k, d:d + 1, :, :],
                                                 zbpv[:, k, d:d + 1, :, :],
                                                 ACTF.Identity, bias=0.0)
                        # finish this pair's denominator + reciprocal on DVE
                        dsl = slice(2 * k * NF, (2 * k + 2) * NF)
                        nc.vector.tensor_tensor(den16[:, dsl], den16[:, dsl],
                                                Ev[:, 7, hs, :], ADD)
                        with nc.allow_low_precision(reason="fp16 r is fine"):
                            nc.vector.reciprocal(r16[:, dsl], den16[:, dsl])
                    # den accumulation on Pool (SBUF fp16), per h-pair chain
                    elif t == 0:
                        nc.gpsimd.tensor_copy(den16[:, 2 * k * NF:(2 * k + 2) * NF],
                                              Ev[:, 0, hs, :])
                    else:
                        dsl = slice(2 * k * NF, (2 * k + 2) * NF)
                        nc.gpsimd.tensor_tensor(den16[:, dsl], den16[:, dsl],
                                                Ev[:, t, hs, :], ADD)

            bridge(N_BRIDGE_TAIL)
            r16v = r16.rearrange("p (h n) -> p h n", h=4)
            zr = wpool.tile([128, 12 * NF], f16, tag="zr")
            zrv = zr.rearrange("p (k d h n) -> p k d h n", k=2, d=3, h=2)

            # PSUM accumulate-groups are clobbered by any start=True on the
            # same bank -> one group per bank: c0/c1 in the two piece bufs,
            # c2 in the (freed) big-pool tile.
            out_ps0 = piecepool.tile([128, 2 * NF], f32, tag="piece")
            out_ps1 = piecepool.tile([128, 2 * NF], f32, tag="piece")
            out_ps2 = pspool.tile([128, 12 * NF], f32, tag="big")

            def out_slot(c):
                return [out_ps0, out_ps1, out_ps2][c][:, 0:NF]

            # fine-grained tail pipeline: per (k,d) scale -> matmuls
            for k in range(2):
                hs = slice(2 * k, 2 * k + 2)
                rbk = r16v[:, hs, :].unsqueeze(0 + 1).broadcast_to((128, 1, 2, NF))
                for d in range(3):
                    nc.vector.tensor_tensor(zrv[:, k, d:d + 1, :, :],
                                            zbv[:, k, d:d + 1, :, :], rbk, MULT)
                    for c in range(3):
                        for h2 in range(2):
                            nc.tensor.matmul(out_slot(c),
                                             wM(2 * k + h2, c, d),
                                             zrv[:, k, d, h2, :],
                                             start=(k == 0 and d == 0 and h2 == 0),
                                             stop=(k == 1 and d == 2 and h2 == 1))
            out16 = wpool.tile([128, 3 * NF], f16, tag="out16")
            odst = o_dram.rearrange("c (p n) -> p c n", p=128)
            for c in range(3):
                nc.scalar.activation(out16[:, c * NF:(c + 1) * NF],
                                     out_slot(c),
                                     ACTF.Identity, bias=consts[:, c:c + 1])
                nc.sync.dma_start(out=odst[:, c:c + 1, :],
                                  in_=out16[:, c * NF:(c + 1) * NF])
            if DEBUG_TAPS:
                nc.sync.dma_start(out=dbg_yq, in_=yq16[:])
                nc.sync.dma_start(out=dbg_e, in_=E[:])
                nc.sync.dma_start(out=dbg_den, in_=r16[:])
                nc.sync.dma_start(out=dbg_zb, in_=zr[:])

    nc.finalize()
    return nc


def _get_program(key):
    if key not in _CACHE:
        _CACHE[key] = _build_program()
    return _CACHE[key]


def _weight_mats_f16(Ghat, ghat, M):
    """61 stacked 128x128 fp16 mats: [0]=I, [1:25]=G'(h=2,3)*I, [25:61]=M*I."""
    eye = np.eye(128, dtype=np.float32)
    mats = np.empty((NMATS, 128, 128), np.float32)
    mats[0] = eye
    Gp = np.empty((2, 3, 4), np.float32)   # (h-2, d, c')
    Gp[:, :, :3] = np.transpose(Ghat[2:], (0, 2, 1))
    Gp[:, :, 3] = ghat[2:]
    mats[1:25] = Gp.reshape(24, 1, 1) * eye
    mats[25:61] = M.reshape(36, 1, 1) * eye
    return np.ascontiguousarray(
        mats.transpose(1, 0, 2).reshape(128, NMATS * 128)).astype(np.float16)


def kernel(z_receive, W_in, b_in, W_q, b_q, W_k, b_k, W_v, b_v, W_o, b_o):
    from concourse.bass_utils import run_bass_kernel_spmd

    z_receive = np.asarray(z_receive, np.float32)
    Ghat, ghat, M, bhat = _fold_weights(W_in, b_in, W_q, b_q, W_k, b_k, W_v, b_v, W_o, b_o)
    global _GHAT, _GSMALL
    _GHAT, _GSMALL = Ghat, ghat
    wm = _weight_mats_f16(Ghat, ghat, M)
    consts = np.zeros((128, 4), np.float32)
    consts[:, 0:3] = bhat

    nc = _get_program("trn2_attn_v3")

    z16 = z_receive.astype(np.float16)
    in_maps = []
    for i in range(NCORES):
        b, hh = i // 2, (i % 2) * 128
        shard = np.ascontiguousarray(
            z16[b, :, :, hh:hh + 128, :]).reshape(24, NPIX)
        in_maps.append({"z": shard, "ident": wm, "consts": consts})

    res = run_bass_kernel_spmd(nc, in_maps, list(range(NCORES)))

    out = np.empty((B, 3, H, W), np.float32)
    for i in range(NCORES):
        b, hh = i // 2, (i % 2) * 128
        out[b, :, hh:hh + 128, :] = res.results[i]["out"].astype(np.float32).reshape(3, 128, W)
    return out


# revision 53
# speedup vs baseline: 1.0572x; 1.0054x over previous
"""Trainium2 Bass kernel for per-pixel temporal attention (nn_Attention).

Reference computation, per pixel (B,H,W independent; T=8, C=3):
  x = Linear_in(z); q,k,v = Linear_{q,k,v}(x); 4-head attention over T,
  take row t=T-1, project to 3 channels.

Only the LAST timestep's attention output is used, so the whole pipeline
folds (host-side, weights only) to per-pixel:
  yq[h,d] = sum_{c'} G'[h,c',d]*z7'[c']          (z7' = [z7; 1], 12 outs)
  s[h,t]  = sum_d yq[h,d]*z[t,d]                 (32)
  e = exp(s); den[h] = sum_t e; r = 1/den
  zbar[h,d] = sum_t e[h,t]*z[t,d]
  out[c] = sum_{h,d} M[h,c,d]*(zbar[h,d]*r[h]) + bhat[c]

Sharding: data-parallel over 8 cores; core i takes batch b=i//2,
row-half i%2 -> a (24, 32768) shard per core; z uploaded as fp16.

Device mapping (pixels-on-partitions: 128 partitions x 256 pixels,
per-pixel features as fp16 planes of 256 on the free axis):
  - products (yq*z, e*z, zbar*r)          -> VectorE fp16 TT (2x mode)
  - yq / s-sum / zbar-sum / M-map         -> TensorE scaled-identity
    matmuls accumulating in PSUM fp32
  - exp, PSUM evictions                   -> ScalarE (ACT)
  - den accumulation                      -> GpSimd (Pool), in SBUF fp16
  - reciprocal                            -> VectorE
Output written fp16, host casts to fp32.
"""

import numpy as np

HEADS, DK = 4, 8
B, H, W = 4, 256, 256
NPIX = 128 * 256          # pixels per core shard
NF = 256                  # pixels per partition
NCORES = 8

_CACHE = {}


def _fold_weights(W_in, b_in, W_q, b_q, W_k, b_k, W_v, b_v, W_o, b_o):
    f8 = np.float64
    W_in, b_in, W_q, b_q, W_k, b_k, W_v, b_v, W_o, b_o = [
        np.asarray(x, f8) for x in (W_in, b_in, W_q, b_q, W_k, b_k, W_v, b_v, W_o, b_o)]
    A_q = W_q @ W_in; c_q = W_q @ b_in + b_q
    A_k = W_k @ W_in; c_k = W_k @ b_in + b_k
    A_v = W_v @ W_in; c_v = W_v @ b_in + b_v
    scale = 1.0 / np.sqrt(DK)
    Ghat = np.zeros((HEADS, 3, 3)); ghat = np.zeros((HEADS, 3)); M = np.zeros((HEADS, 3, 3))
    for h in range(HEADS):
        sl = slice(h * DK, (h + 1) * DK)
        Ghat[h] = A_q[sl].T @ A_k[sl] * scale
        ghat[h] = A_k[sl].T @ c_q[sl] * scale
        M[h] = W_o[:, sl] @ A_v[sl]
    bhat = W_o @ c_v + b_o
    return (Ghat.astype(np.float32), ghat.astype(np.float32),
            M.astype(np.float32), bhat.astype(np.float32))


# Stacked 128x128 fp16 weight mats: [0]=I; [1+((h-2)*3+d)*4+c'] =
# G'[h,c',d]*I for h=2,3 only (heads 0,1 compute yq on DVE with scalar
# immediates); [25+h*9+c*3+d] = M[h,c,d]*I.
NMATS = 61

# folded weights baked into the program as immediates (set by kernel())
_GHAT = None
_GSMALL = None

# PE p-state bridge sizes (64-col dummy matmuls across known PE idle
# windows; tuned against the cost-model timeline)
N_BRIDGE_HEAD = 40
N_BRIDGE_TAIL = 0
DEBUG_TAPS = False


def _build_program():
    import concourse.bass as bass
    import concourse.tile as tile
    from concourse import bacc, mybir

    f32, f16 = mybir.dt.float32, mybir.dt.float16
    MULT, ADD = mybir.AluOpType.mult, mybir.AluOpType.add
    ACTF = mybir.ActivationFunctionType

    nc = bacc.Bacc("TRN2", target_bir_lowering=False, debug=False)
    z_dram = nc.dram_tensor("z", [24, NPIX], f16, kind="ExternalInput").ap()
    i_dram = nc.dram_tensor("ident", [128, NMATS * 128], f16, kind="ExternalInput").ap()
    c_dram = nc.dram_tensor("consts", [128, 4], f32, kind="ExternalInput").ap()
    o_dram = nc.dram_tensor("out", [3, NPIX], f16, kind="ExternalOutput").ap()
    if DEBUG_TAPS:
        dbg_yq = nc.dram_tensor("dbg_yq", [128, 12 * NF], f16, kind="ExternalOutput").ap()
        dbg_e = nc.dram_tensor("dbg_e", [128, 32 * NF], f16, kind="ExternalOutput").ap()
        dbg_den = nc.dram_tensor("dbg_den", [128, 4 * NF], f16, kind="ExternalOutput").ap()
        dbg_zb = nc.dram_tensor("dbg_zb", [128, 12 * NF], f16, kind="ExternalOutput").ap()

    with tile.TileContext(nc) as tc:
        with (
            tc.tile_pool(name="const", bufs=1) as cpool,
            tc.tile_pool(name="z16", bufs=1) as z16pool,
            tc.tile_pool(name="work", bufs=1) as wpool,
            tc.tile_pool(name="prod", bufs=1) as ppool,
            tc.tile_pool(name="psum", bufs=1, space="PSUM") as pspool,
            tc.tile_pool(name="piece", bufs=2, space="PSUM") as piecepool,
        ):
            wmats = cpool.tile([128, NMATS * 128], f16)
            ident = wmats[:, 0:128]

            def wG(h, d, cp):   # G'[h,c',d] * I   (h = 2 or 3)
                j = 1 + ((h - 2) * 3 + d) * 4 + cp
                return wmats[:, j * 128:(j + 1) * 128]

            def wM(h, c, d):    # M[h,c,d] * I
                j = 25 + h * 9 + c * 3 + d
                return wmats[:, j * 128:(j + 1) * 128]

            zsrc = z_dram.rearrange("(t c) (p n) -> p t c n", t=8, c=3, p=128)
            z16 = z16pool.tile([128, 24 * NF], f16)
            z16v = z16.rearrange("p (t c n) -> p t c n", t=8, c=3)

            # ---- DMA order: ident+z7 first, then G' mats in per-h chunks,
            #      then the rest of z, then M mats; consts (only needed at
            #      the end) last. Two issue queues overlap per-DMA setup.
            nc.sync.dma_start(out=wmats[:, 0:128], in_=i_dram[:, 0:128])
            nc.scalar.dma_start(out=z16v[:, 7, :, :], in_=zsrc[:, 7, :, :])
            for hh in range(2):
                lo, hi = (1 + hh * 12) * 128, (13 + hh * 12) * 128
                nc.sync.dma_start(out=wmats[:, lo:hi], in_=i_dram[:, lo:hi])
            nc.scalar.dma_start(out=z16v[:, 0:4, :, :], in_=zsrc[:, 0:4, :, :])
            nc.scalar.dma_start(out=z16v[:, 4:7, :, :], in_=zsrc[:, 4:7, :, :])
            nc.sync.dma_start(out=wmats[:, 25 * 128:], in_=i_dram[:, 25 * 128:])
            consts = cpool.tile([128, 4], f32)
            nc.scalar.dma_start(out=consts[:], in_=c_dram)

            # ones plane for the ghat (c'=3) yq terms
            ones = wpool.tile([128, NF], f16, tag="ones")
            nc.gpsimd.memset(ones[:], 1.0)

            def bridge(n):
                """Keep the PE p-state ramp alive across known idle windows:
                tiny matmuls into a rotating piece, no data deps beyond ident."""
                for _ in range(n):
                    dps = piecepool.tile([128, 2 * NF], f32, tag="piece")
                    nc.tensor.matmul(dps[:, 0:64], ident,
                                     ident[:, 0:64], start=True, stop=True)

            bridge(N_BRIDGE_HEAD)

            # ---- yq[h,d] = sum_c Ghat*z7[c] + ghat: heads 0,1 on DVE
            #      (tensor_scalar 4x with immediates, no weight-DMA wait) so
            #      the t-loop starts at z7-arrival; heads 2,3 on PE+ACT in
            #      parallel via the scaled-identity route.
            z7p = [z16v[:, 7, c, :] for c in range(3)]
            yq16 = wpool.tile([128, 12 * NF], f16, tag="yq16")
            yqv = yq16.rearrange("p (h d n) -> p h d n", h=4, d=3)
            sz = wpool.tile([128, 9 * NF], f16, tag="sz")
            szv = sz.rearrange("p (c d n) -> p c d n", c=3, d=3)
            tmp = wpool.tile([128, 3 * NF], f16, tag="yqtmp")
            tmpv = tmp.rearrange("p (d n) -> p d n", d=3)
            for h in range(2):
                for c in range(3):
                    for d in range(3):
                        s2 = float(_GSMALL[h, d]) if c == 0 else 0.0
                        nc.vector.tensor_scalar(
                            szv[:, c, d, :], z7p[c],
                            float(_GHAT[h, c, d]), s2, MULT, ADD)
                nc.vector.tensor_tensor(tmpv[:, :, :], szv[:, 0, :, :],
                                        szv[:, 1, :, :], ADD)
                nc.vector.tensor_tensor(yqv[:, h, :, :], tmpv[:, :, :],
                                        szv[:, 2, :, :], ADD)
            z7q = z7p + [ones[:]]
            yq_ps = pspool.tile([128, 12 * NF], f32, tag="big")
            for h in (2, 3):
                for d in range(3):
                    j = h * 3 + d
                    for cp in range(4):
                        nc.tensor.matmul(yq_ps[:, j * NF:(j + 1) * NF],
                                         wG(h, d, cp), z7q[cp],
                                         start=(cp == 0), stop=(cp == 3))
            nc.scalar.activation(yq16[:, 6 * NF:], yq_ps[:, 6 * NF:],
                                 ACTF.Identity, bias=0.0)
            yqv = yq16.rearrange("p (h d n) -> p h d n", h=4, d=3)

            # ---- t-pipelined middle: P (DVE) -> s (PE) -> exp (ACT)
            #      -> P2 (DVE) -> zbar accumulation (PE), den accum (Pool)
            P = ppool.tile([128, 96 * NF], f16, tag="P")
            Pv = P.rearrange("p (t d h n) -> p t d h n", t=8, d=3, h=4)
            P2 = ppool.tile([128, 96 * NF], f16, tag="P2")
            # (t, hpair, d, h2, n): the (d,h2) slab per (t,hpair) is contiguous
            P2v = P2.rearrange("p (t k d h n) -> p t k d h n", t=8, k=2, d=3, h=2)
            E = wpool.tile([128, 32 * NF], f16, tag="E")
            Ev = E.rearrange("p (t h n) -> p t h n", t=8, h=4)
            den16 = wpool.tile([128, 4 * NF], f16, tag="den16")
            r16 = wpool.tile([128, 4 * NF], f16, tag="r16")
            zb_ps = pspool.tile([128, 12 * NF], f32, tag="big")
            # layout (k, d, h2): the per-h-pair slab [p, k] is contiguous
            zbpv = zb_ps.rearrange("p (k d h n) -> p k d h n", k=2, d=3, h=2)
            zb16 = wpool.tile([128, 12 * NF], f16, tag="zb16")
            zbv = zb16.rearrange("p (k d h n) -> p k d h n", k=2, d=3, h=2)
            yb = yqv.transpose([0, 2, 1, 3]).unsqueeze(1).broadcast_to((128, 1, 3, 4, NF))

            for t in range(8):
                zbt = z16v[:, t:t + 1, :, :].unsqueeze(3).broadcast_to((128, 1, 3, 4, NF))
                for k in range(2):
                    hs = slice(2 * k, 2 * k + 2)
                    # P[t,d,h-half] = yq[h,d] * z[t,d]   (DVE fp16 2x)
                    nc.vector.tensor_tensor(Pv[:, t:t + 1, :, hs, :],
                                            yb[:, :, :, hs, :],
                                            zbt[:, :, :, 0:2, :], MULT)
                    # s piece (h-pair) -> exp
                    s_ps = piecepool.tile([128, 2 * NF], f32, tag="piece")
                    for d in range(3):
                        nc.tensor.matmul(s_ps[:], ident,
                                         Pv[:, t, d, hs, :],
                                         start=(d == 0), stop=(d == 2))
                    nc.scalar.activation(Ev[:, t, hs, :],
                                         s_ps.rearrange("p (j n) -> p j n", j=2),
                                         ACTF.Exp)
                    # P2[t,k,d,h2] = e[t,h] * z[t,d]   (DVE fp16 2x)
                    ebt = Ev[:, t:t + 1, hs, :].unsqueeze(2).broadcast_to((128, 1, 3, 2, NF))
                    nc.vector.tensor_tensor(P2v[:, t:t + 1, k, :, :, :], ebt,
                                            zbt[:, :, :, 0:2, :], MULT)
                    # zbar accumulation: contiguous 2-plane (one-bank) pieces
                    off = (t * 2 + k) * 6 * NF
                    for q in range(3):
                        dst = (k * 6 + q * 2) * NF
                        nc.tensor.matmul(zb_ps[:, dst:dst + 2 * NF], ident,
                                         P2[:, off + q * 2 * NF:off + (q + 1) * 2 * NF],
                                         start=(t == 0), stop=(t == 7))
                    if t == 7:
                        # this h-pair's zbar groups just stopped: evict now so
                        # the tail's zr/Mout chain starts as early as possible
                        for d in range(3):
                            nc.scalar.activation(zbv[:, k, d:d + 1, :, :],
                                                 zbpv[:, k, d:d + 1, :, :],
                                                 ACTF.Identity, bias=0.0)
                        # finish this pair's denominator + reciprocal on DVE
                        dsl = slice(2 * k * NF, (2 * k + 2) * NF)
                        nc.vector.tensor_tensor(den16[:, dsl], den16[:, dsl],
                                                Ev[:, 7, hs, :], ADD)
                        with nc.allow_low_precision(reason="fp16 r is fine"):
                            nc.vector.reciprocal(r16[:, dsl], den16[:, dsl])
                    # den accumulation on Pool (SBUF fp16), per h-pair chain
                    elif t == 0:
                        nc.gpsimd.tensor_copy(den16[:, 2 * k * NF:(2 * k + 2) * NF],
                                              Ev[:, 0, hs, :])
                    else:
                        dsl = slice(2 * k * NF, (2 * k + 2) * NF)
                        nc.gpsimd.tensor_tensor(den16[:, dsl], den16[:, dsl],
                                                Ev[:, t, hs, :], ADD)

            bridge(N_BRIDGE_TAIL)
            r16v = r16.rearrange("p (h n) -> p h n", h=4)
            zr = wpool.tile([128, 12 * NF], f16, tag="zr")
            zrv = zr.rearrange("p (k d h n) -> p k d h n", k=2, d=3, h=2)

            # PSUM accumulate-groups are clobbered by any start=True on the
            # same bank -> one group per bank: c0/c1 in the two piece bufs,
            # c2 in the (freed) big-pool tile.
            out_ps0 = piecepool.tile([128, 2 * NF], f32, tag="piece")
            out_ps1 = piecepool.tile([128, 2 * NF], f32, tag="piece")
            out_ps2 = pspool.tile([128, 12 * NF], f32, tag="big")

            def out_slot(c):
                return [out_ps0, out_ps1, out_ps2][c][:, 0:NF]

            # fine-grained tail pipeline: per (k,d) scale -> matmuls
            for k in range(2):
                hs = slice(2 * k, 2 * k + 2)
                rbk = r16v[:, hs, :].unsqueeze(0 + 1).broadcast_to((128, 1, 2, NF))
                for d in range(3):
                    nc.vector.tensor_tensor(zrv[:, k, d:d + 1, :, :],
                                            zbv[:, k, d:d + 1, :, :], rbk, MULT)
                    for c in range(3):
                        for h2 in range(2):
                            nc.tensor.matmul(out_slot(c),
                                             wM(2 * k + h2, c, d),
                                             zrv[:, k, d, h2, :],
                                             start=(k == 0 and d == 0 and h2 == 0),
                                             stop=(k == 1 and d == 2 and h2 == 1))
            out16 = wpool.tile([128, 3 * NF], f16, tag="out16")
            odst = o_dram.rearrange("c (p n) -> p c n", p=128)
            dmaq = [nc.sync, nc.scalar, nc.gpsimd]
            for c in range(3):
                nc.scalar.activation(out16[:, c * NF:(c + 1) * NF],
                                     out_slot(c),
                                     ACTF.Identity, bias=consts[:, c:c + 1])
                dmaq[c].dma_start(out=odst[:, c:c + 1, :],
                                  in_=out16[:, c * NF:(c + 1) * NF])
            if DEBUG_TAPS:
                nc.sync.dma_start(out=dbg_yq, in_=yq16[:])
                nc.sync.dma_start(out=dbg_e, in_=E[:])
                nc.sync.dma_start(out=dbg_den, in_=r16[:])
                nc.sync.dma_start(out=dbg_zb, in_=zr[:])

    nc.finalize()
    return nc


def _get_program(key):
    if key not in _CACHE:
        _CACHE[key] = _build_program()
    return _CACHE[key]


def _weight_mats_f16(Ghat, ghat, M):
    """61 stacked 128x128 fp16 mats: [0]=I, [1:25]=G'(h=2,3)*I, [25:61]=M*I."""
    eye = np.eye(128, dtype=np.float32)
    mats = np.empty((NMATS, 128, 128), np.float32)
    mats[0] = eye
    Gp = np.empty((2, 3, 4), np.float32)   # (h-2, d, c')
    Gp[:, :, :3] = np.transpose(Ghat[2:], (0, 2, 1))
    Gp[:, :, 3] = ghat[2:]
    mats[1:25] = Gp.reshape(24, 1, 1) * eye
    mats[25:61] = M.reshape(36, 1, 1) * eye
    return np.ascontiguousarray(
        mats.transpose(1, 0, 2).reshape(128, NMATS * 128)).astype(np.float16)


def kernel(z_receive, W_in, b_in, W_q, b_q, W_k, b_k, W_v, b_v, W_o, b_o):
    from concourse.bass_utils import run_bass_kernel_spmd

    z_receive = np.asarray(z_receive, np.float32)
    Ghat, ghat, M, bhat = _fold_weights(W_in, b_in, W_q, b_q, W_k, b_k, W_v, b_v, W_o, b_o)
    global _GHAT, _GSMALL
    _GHAT, _GSMALL = Ghat, ghat
    wm = _weight_mats_f16(Ghat, ghat, M)
    consts = np.zeros((128, 4), np.float32)
    consts[:, 0:3] = bhat

    nc = _get_program("trn2_attn_v3")

    z16 = z_receive.astype(np.float16)
    in_maps = []
    for i in range(NCORES):
        b, hh = i // 2, (i % 2) * 128
        shard = np.ascontiguousarray(
            z16[b, :, :, hh:hh + 128, :]).reshape(24, NPIX)
        in_maps.append({"z": shard, "ident": wm, "consts": consts})

    res = run_bass_kernel_spmd(nc, in_maps, list(range(NCORES)))

    out = np.empty((B, 3, H, W), np.float32)
    for i in range(NCORES):
        b, hh = i // 2, (i % 2) * 128
        out[b, :, hh:hh + 128, :] = res.results[i]["out"].astype(np.float32).reshape(3, 128, W)
    return out
